# revision 18
# baseline (speedup 1.0000x reference)
"""Trainium2 Bass kernel for nn_Attention_53455162966555.

Multi-head attention block: B=8, N=1024, DIM=1024, H=16 heads, hd=64.
Sharding: data-parallel over batch - core b computes x[b] with full weights
on NeuronCore b; no collectives.

v2 design (vs the f32r v1 at 292us):
  - All QKV projections run as fp8(e4m3) DoubleRow matmuls with a 3-term
    residual expansion (x8@w8 + rx@w8 + x8@rw): K=256 per instruction at
    0.5 cycles/row -> 0.75x the f32r cycle count with ~f32r-level accuracy
    (MC rel err 3.5e-3 vs 2e-2 gate). Host pre-quantizes x^T and w_qkv
    (scales 16 and 512, both powers of 2) plus their residuals, so the
    device does no quantization work and no PE transposes of x.
  - S = q.k^T stays f32r (fp8 scores fail accuracy), computed per head-pair
    with tile_position row-packing as in v1, but into [128,1024] 2-bank
    PSUM tiles so exp runs as one 1024-wide activation (amortizes the
    PSUM-access overhead).
  - PV uses the "O-form": O[q,d] = sum_k P^T[k,q-chunk]^T V[k,d] with the
    128-wide q chunk as stationary and the 65-wide V (ones column appended
    for the softmax denominator) as moving operand, in bf16: 8x65 moving
    rows per (head, q-tile) instead of 1024 -> half the f32r O^T-form cost.
  - Softmax normalize becomes a [128,1] reciprocal + per-partition-scalar
    multiply (DVE+Pool), then O head-pairs are PE-transposed (bf16 identity,
    1 cycle/row) straight into the projection layout OT[c'=h*64+d, n].
  - Final projection in bf16 from OT with host-permuted bf16 w_proj
    (c = d*16+h -> c' = h*64+d undoes the reference's [B,N,hd,H] interleave).
  - b_proj applied host-side (exact; zero for this model).
"""

import numpy as np
import ml_dtypes

import concourse.bass as bass
import concourse.mybir as mybir
import concourse.tile as tile
from concourse import bacc
from concourse.masks import make_identity

P = 128
DIM = 1024
H = 16
HD = 64
F3 = 3 * DIM
CS = DIM // P
SCALE = HD ** -0.5
SX = 16.0       # fp8 scale for x
SW = 512.0      # fp8 scale for w_qkv
INV_SXSW = 1.0 / (SX * SW)

FP32 = mybir.dt.float32
FP32R = mybir.dt.float32r
BF16 = mybir.dt.bfloat16
E4 = mybir.dt.float8e4
Exp = mybir.ActivationFunctionType.Exp
DR = mybir.MatmulPerfMode.DoubleRow


def build_nc(N=1024):
    NT = N // P

    nc = bacc.Bacc(None, target_bir_lowering=False)
    with tile.TileContext(nc) as tc:
        with tc.tile_pool(name="dram", bufs=1, space="DRAM") as dram:
            xT8_d = dram.tile([DIM, N], E4, kind="ExternalInput")
            rxT8_d = dram.tile([DIM, N], E4, kind="ExternalInput")
            w8_d = dram.tile([DIM, F3], E4, kind="ExternalInput")
            rw8_d = dram.tile([DIM, F3], E4, kind="ExternalInput")
            wp_d = dram.tile([DIM, DIM], BF16, kind="ExternalInput")
            y_d = dram.tile([N, DIM], FP32, kind="ExternalOutput")
            _build_core(nc, tc, xT8_d, rxT8_d, w8_d, rw8_d, wp_d, y_d, N, NT)
    nc.compile()
    names = dict(xT8=xT8_d.name, rxT8=rxT8_d.name, w8=w8_d.name,
                 rw8=rw8_d.name, wp=wp_d.name, y=y_d.name)
    return nc, names


def _build_core(nc, tc, xT8_d, rxT8_d, w8_d, rw8_d, wp_d, y_d, N, NT):
    xT8_r = xT8_d[:].rearrange("(cs p) n -> p cs n", p=P)
    rxT8_r = rxT8_d[:].rearrange("(cs p) n -> p cs n", p=P)
    w8_r = w8_d[:].rearrange("(cs p) f -> p cs f", p=P)
    rw8_r = rw8_d[:].rearrange("(cs p) f -> p cs f", p=P)
    wp_r = wp_d[:].rearrange("(cs p) f -> p cs f", p=P)
    y_r = y_d[:].rearrange("(nt p) f -> p nt f", p=P)

    with (
        tc.tile_pool(name="consts", bufs=1) as consts,
        tc.tile_pool(name="persist", bufs=1) as persist,
        tc.tile_pool(name="qkt", bufs=2) as qkt_pool,
        tc.tile_pool(name="est", bufs=1) as est_pool,
        tc.tile_pool(name="onorm", bufs=16) as onorm_pool,
        tc.tile_pool(name="rden", bufs=4) as rden_pool,
        tc.tile_pool(name="ysb", bufs=2) as ysb_pool,
        tc.tile_pool(name="psum", bufs=1, space="PSUM") as psum,
    ):
        identb = consts.tile([P, P], BF16)
        make_identity(nc, identb[:])

        # persistent SBUF tensors
        xT8 = persist.tile([P, CS, N], E4)
        rxT8 = persist.tile([P, CS, N], E4)
        w8qk = persist.tile([P, CS, 2 * DIM], E4)
        rw8qk = persist.tile([P, CS, 2 * DIM], E4)
        wv8 = persist.tile([P, CS, DIM], E4)
        rwv8 = persist.tile([P, CS, DIM], E4)
        wp_sb = persist.tile([P, CS, DIM], BF16)
        V_sb = persist.tile([P, NT, H, HD + 1], BF16)
        OT = persist.tile([P, CS, N], BF16)

        # DMA order = first-use order: qk(0) runs first (xT8 + hp0-3 q/k
        # weight chunks + residuals), V' units follow as iteration-0 fill
        # (wv8), the other q/k chunks serve hp4-7, wp only feeds the tail.
        for cp in range(4):
            cs2 = slice(2 * cp, 2 * cp + 2)
            nc.sync.dma_start(xT8[:, cs2, :], xT8_r[:, cs2, :])
        for sl in (slice(0, 512), slice(DIM, DIM + 512)):
            nc.sync.dma_start(w8qk[:, :, sl], w8_r[:, :, sl])
        for cp in range(4):
            cs2 = slice(2 * cp, 2 * cp + 2)
            nc.sync.dma_start(rxT8[:, cs2, :], rxT8_r[:, cs2, :])
        for sl in (slice(0, 512), slice(DIM, DIM + 512)):
            nc.sync.dma_start(rw8qk[:, :, sl], rw8_r[:, :, sl])
        for cp in range(4):
            cs2 = slice(2 * cp, 2 * cp + 2)
            nc.sync.dma_start(wv8[:, cs2, :], w8_r[:, cs2, 2 * DIM:F3])
            nc.sync.dma_start(rwv8[:, cs2, :], rw8_r[:, cs2, 2 * DIM:F3])
        for sl in (slice(512, DIM), slice(DIM + 512, 2 * DIM)):
            nc.sync.dma_start(w8qk[:, :, sl], w8_r[:, :, sl])
            nc.sync.dma_start(rw8qk[:, :, sl], rw8_r[:, :, sl])
        nc.sync.dma_start(wp_sb[:], wp_r[:])
        nc.vector.memset(V_sb[:, :, :, HD:HD + 1], 1.0)

        # ---- V' = x @ Wv (+ones col), fp8 3-term -----------------------------
        # emitted as fill units inside the first iterations; term order
        # (x,w)x4, (rx,w)x4, (x,rw)x4 matches the DMA arrival order
        def vprime_unit(fc, nt):
            pv = psum.tile([P, 512], FP32, tag="u", bufs=2,
                           name=f"pv_{fc}_{nt}")
            xa_nt = xT8[:, :, nt * P:(nt + 1) * P]
            rx_nt = rxT8[:, :, nt * P:(nt + 1) * P]
            for g in range(2):
                out = pv[:, g * 256:(g + 1) * 256]
                f0 = fc * 512 + g * 256
                first = True
                for xa, wa in ((xa_nt, wv8), (rx_nt, wv8),
                               (xa_nt, rwv8)):
                    for i in range(4):
                        cp = slice(2 * i, 2 * i + 2)
                        nc.tensor.matmul(
                            out, xa[:, cp, :], wa[:, cp, f0:f0 + 256],
                            start=first,
                            stop=(i == 3 and wa is rwv8),
                            perf_mode=DR,
                        )
                        first = False
            nc.vector.tensor_scalar_mul(
                V_sb[:, nt, fc * 8:(fc + 1) * 8, 0:HD],
                pv[:, :].rearrange("p (h d) -> p h d", d=HD),
                INV_SXSW,
            )

        vprime_units = [(3072, (lambda fc_, nt_: lambda: vprime_unit(fc_, nt_))(fc, nt))
                        for fc in range(2) for nt in range(NT)]

        # ---- q/k projection (fp8 3-term), one psum tile per unit -----------
        def qk_half(hp, qk_t, qi, qc, g, state):
            ft = hp if qi == 0 else CS + hp
            if g == 0:
                state["pqk"] = psum.tile([P, 512], FP32, tag="u", bufs=2,
                                         name=f"pqk_{hp}_{qi}_{qc}")
            pqk = state["pqk"]
            out = pqk[:, g * 256:(g + 1) * 256]
            n0 = qc * 512 + g * 256
            first = True
            for wa, xa in ((w8qk, xT8), (w8qk, rxT8),
                           (rw8qk, xT8)):
                for i in range(4):
                    cp = slice(2 * i, 2 * i + 2)
                    nc.tensor.matmul(
                        out, wa[:, cp, ft * P:(ft + 1) * P],
                        xa[:, cp, n0:n0 + 256],
                        start=first,
                        stop=(i == 3 and wa is rw8qk),
                        perf_mode=DR,
                    )
                    first = False
            if g == 1:
                nc.vector.tensor_scalar_mul(
                    qk_t[:, qi, qc * 512:(qc + 1) * 512], pqk[:],
                    INV_SXSW)

        def qk_unit(hp, qk_t, qi, qc):
            state = {}
            for g in range(2):
                qk_half(hp, qk_t, qi, qc, g, state)

        def emit_qk_proj(hp):
            qk_t = qkt_pool.tile([P, 2, N], FP32R, tag="qkt",
                                 name=f"qk_t_{hp}")
            for qi in range(2):
                for qc in range(2):
                    qk_unit(hp, qk_t, qi, qc)
            return qk_t

        def s_unit(hp, qk_t, est, hi, kt):
            po = hi * HD
            ps = psum.tile([P, N], FP32, tag="s", bufs=2,
                           name=f"ps_{hp}_{hi}_{kt}")
            lhsT = qk_t[po:po + HD, 1, kt * P:(kt + 1) * P]
            for qc in range(2):
                nc.tensor.matmul(
                    ps[:, qc * 512:(qc + 1) * 512],
                    lhsT,
                    qk_t[po:po + HD, 0, qc * 512:(qc + 1) * 512],
                    start=True, stop=True,
                    tile_position=(po, 0),
                )
            nc.scalar.activation(est[hi][:, kt, :], ps[:], Exp, scale=SCALE)

        def make_est(hp):
            return [est_pool.tile([P, NT, N], BF16, tag=f"est{hi}",
                                  bufs=(2 if hi == 0 else 1),
                                  name=f"est_{hp}_{hi}")
                    for hi in range(2)]

        def pv_unit(hp, est, hi, qt, ons):
            h = 2 * hp + hi
            oacc = psum.tile([P, HD + 1], FP32, tag="o", bufs=2,
                             name=f"oacc_{hp}_{hi}_{qt}")
            for kt in range(NT):
                nc.tensor.matmul(
                    oacc[:],
                    est[hi][:, kt, qt * P:(qt + 1) * P],
                    V_sb[:, kt, h, :],
                    start=(kt == 0), stop=(kt == NT - 1),
                    skip_group_check=True,
                )
            rd = rden_pool.tile([P, 1], FP32, tag="rd",
                                name=f"rd_{hp}_{hi}_{qt}")
            nc.vector.reciprocal(rd[:], oacc[:, HD:HD + 1])
            nc.gpsimd.tensor_scalar_mul(
                ons[qt][:, hi, :], oacc[:, 0:HD], rd[:])

        def tr_unit(hp, ons, qt):
            tr = psum.tile([P, P], BF16, tag="o", bufs=2,
                           name=f"tr_{hp}_{qt}")
            nc.tensor.transpose(tr[:], ons[qt][:], identb[:])
            nc.gpsimd.tensor_copy(OT[:, hp, qt * P:(qt + 1) * P], tr[:])

        def make_ons(hp):
            return [onorm_pool.tile([P, 2, HD], BF16, tag="on",
                                    name=f"on_{hp}_{qt}")
                    for qt in range(NT)]

        def interleave(s_emits, fill):
            """Emit the 16 S+exp slots of an iteration, draining ~1467 PE
            cycles of fill work between consecutive slots so the PE stays
            busy while exps free the S psum buffers at their own pace."""
            SLOT = 1467.0
            fi = 0
            debt = SLOT
            for k, su in enumerate(s_emits):
                while fi < len(fill) and debt > 0:
                    c, f = fill[fi]
                    f()
                    debt -= c
                    fi += 1
                su()
                debt += SLOT
            for c, f in fill[fi:]:
                f()

        # Pipeline: iteration hp interleaves S(hp)+exp(hp) slots with
        # PV/transpose work for hp-1 and the q/k projection for hp+1.
        import functools
        qk_t = emit_qk_proj(0)
        est_prev = ons_prev = None
        for hp in range(CS):
            est = make_est(hp)
            s_emits = [functools.partial(s_unit, hp, qk_t, est, hi, kt)
                       for hi in range(2) for kt in range(NT)]
            fill = []
            if hp >= 1:
                ons = make_ons(hp - 1)
                fill += [(520, functools.partial(pv_unit, hp - 1, est_prev,
                                                 hi, qt, ons))
                         for hi in range(2) for qt in range(NT)]
            if hp + 1 < CS:
                qk_next = qkt_pool.tile([P, 2, N], FP32R, tag="qkt",
                                        name=f"qk_t_{hp + 1}")
                qk_states = {(qi, qc): {} for qi in range(2)
                             for qc in range(2)}
                fill += [(1536, functools.partial(qk_half, hp + 1, qk_next,
                                                  qi, qc, g,
                                                  qk_states[(qi, qc)]))
                         for qi in range(2) for qc in range(2)
                         for g in range(2)]
            else:
                qk_next = None
            if hp == 0:
                fill += vprime_units
            if hp >= 1:
                fill += [(128, functools.partial(tr_unit, hp - 1, ons, qt))
                         for qt in range(NT)]
            if hp == CS - 1:
                ons_last = make_ons(hp)
                fill += [(520, functools.partial(pv_unit, hp, est, 0, qt,
                                                 ons_last))
                         for qt in range(NT)]
            interleave(s_emits, fill)
            est_prev = est
            qk_t = qk_next
        for qt in range(NT):
            pv_unit(CS - 1, est_prev, 1, qt, ons_last)
        tr_unit(CS - 1, ons_last, 0)
        ons_prev = ons_last

        # ---- y = OT^T @ wp (bf16) -------------------------------------------
        # chained with the last head-pair's transposes: proj(nt) only needs
        # OT[:, 7, nt-chunk], so each tr(7, qt) immediately unblocks one
        # projection tile. 2-bank psum per nt (tag "s" is idle by now).
        for nt in range(NT):
            if nt + 1 < NT:
                tr_unit(CS - 1, ons_prev, nt + 1)
            py = psum.tile([P, DIM], FP32, tag="s", bufs=2,
                           name=f"py_{nt}")
            for cs in range(CS):
                lhsT = OT[:, cs, nt * P:(nt + 1) * P]
                for fc in range(2):
                    nc.tensor.matmul(
                        py[:, fc * 512:(fc + 1) * 512],
                        lhsT, wp_sb[:, cs, fc * 512:(fc + 1) * 512],
                        start=(cs == 0), stop=(cs == CS - 1),
                    )
            y_sb = ysb_pool.tile([P, DIM], FP32, tag="ysb",
                                 name=f"y_sb_{nt}")
            for fc in range(2):
                nc.vector.tensor_copy(y_sb[:, fc * 512:(fc + 1) * 512],
                                      py[:, fc * 512:(fc + 1) * 512])
                nc.sync.dma_start(y_r[:, nt, fc * 512:(fc + 1) * 512],
                                  y_sb[:, fc * 512:(fc + 1) * 512])


_CACHE = {}


def _get_nc(N=1024):
    if N not in _CACHE:
        _CACHE[N] = build_nc(N)
    return _CACHE[N]


E4NP = ml_dtypes.float8_e4m3


def _prep_weights(w_qkv, w_proj):
    w_qkv = np.asarray(w_qkv, np.float32)
    w_proj = np.asarray(w_proj, np.float32)
    w8 = (w_qkv * SW).astype(E4NP)
    rw8 = (w_qkv * SW - w8.astype(np.float32)).astype(E4NP)
    # permute rows c = d*16+h -> c' = h*64+d to undo the reference's
    # [B,N,hd,H] output interleave
    wp_perm = np.ascontiguousarray(
        w_proj.reshape(HD, H, DIM).transpose(1, 0, 2).reshape(DIM, DIM))
    wp_bf = wp_perm.astype(ml_dtypes.bfloat16)
    return np.ascontiguousarray(w8), np.ascontiguousarray(rw8), wp_bf


def kernel(x, w_qkv, w_proj, b_proj):
    """Full inputs in, full output out. Shards batch across 8 cores."""
    from concourse.bass_utils import run_bass_kernel_spmd

    B, N, C = x.shape
    assert (B, C) == (8, DIM)
    nc, nm = _get_nc(N)
    x = np.asarray(x, dtype=np.float32)
    w8, rw8, wp_bf = _prep_weights(w_qkv, w_proj)
    b_proj_np = np.asarray(b_proj, dtype=np.float32).reshape(1, DIM)

    in_maps = []
    for b in range(B):
        xT = np.ascontiguousarray(x[b].T)
        xT8 = (xT * SX).astype(E4NP)
        rxT8 = (xT * SX - xT8.astype(np.float32)).astype(E4NP)
        in_maps.append({nm["xT8"]: xT8, nm["rxT8"]: rxT8, nm["w8"]: w8,
                       nm["rw8"]: rw8, nm["wp"]: wp_bf})
    res = run_bass_kernel_spmd(nc, in_maps, core_ids=list(range(8)))
    y = np.stack([res.results[b][nm["y"]] for b in range(B)], axis=0)
    if np.any(b_proj_np):
        # exact host-side bias add; no-op for the zero bias this model ships
        y = (y + b_proj_np.reshape(1, 1, DIM)).astype(np.float32)
    return y


# revision 19
# speedup vs baseline: 1.0114x; 1.0114x over previous
"""Trainium2 Bass kernel for nn_Attention_53455162966555.

Multi-head attention block: B=8, N=1024, DIM=1024, H=16 heads, hd=64.
Sharding: data-parallel over batch - core b computes x[b] with full weights
on NeuronCore b; no collectives.

v2 design (vs the f32r v1 at 292us):
  - All QKV projections run as fp8(e4m3) DoubleRow matmuls with a 3-term
    residual expansion (x8@w8 + rx@w8 + x8@rw): K=256 per instruction at
    0.5 cycles/row -> 0.75x the f32r cycle count with ~f32r-level accuracy
    (MC rel err 3.5e-3 vs 2e-2 gate). Host pre-quantizes x^T and w_qkv
    (scales 16 and 512, both powers of 2) plus their residuals, so the
    device does no quantization work and no PE transposes of x.
  - S = q.k^T stays f32r (fp8 scores fail accuracy), computed per head-pair
    with tile_position row-packing as in v1, but into [128,1024] 2-bank
    PSUM tiles so exp runs as one 1024-wide activation (amortizes the
    PSUM-access overhead).
  - PV uses the "O-form": O[q,d] = sum_k P^T[k,q-chunk]^T V[k,d] with the
    128-wide q chunk as stationary and the 65-wide V (ones column appended
    for the softmax denominator) as moving operand, in bf16: 8x65 moving
    rows per (head, q-tile) instead of 1024 -> half the f32r O^T-form cost.
  - Softmax normalize becomes a [128,1] reciprocal + per-partition-scalar
    multiply (DVE+Pool), then O head-pairs are PE-transposed (bf16 identity,
    1 cycle/row) straight into the projection layout OT[c'=h*64+d, n].
  - Final projection in bf16 from OT with host-permuted bf16 w_proj
    (c = d*16+h -> c' = h*64+d undoes the reference's [B,N,hd,H] interleave).
  - b_proj applied host-side (exact; zero for this model).
"""

import numpy as np
import ml_dtypes

import concourse.bass as bass
import concourse.mybir as mybir
import concourse.tile as tile
from concourse import bacc
from concourse.masks import make_identity

P = 128
DIM = 1024
H = 16
HD = 64
F3 = 3 * DIM
CS = DIM // P
SCALE = HD ** -0.5
SX = 16.0       # fp8 scale for x
SW = 512.0      # fp8 scale for w_qkv
INV_SXSW = 1.0 / (SX * SW)

FP32 = mybir.dt.float32
FP32R = mybir.dt.float32r
BF16 = mybir.dt.bfloat16
E4 = mybir.dt.float8e4
Exp = mybir.ActivationFunctionType.Exp
DR = mybir.MatmulPerfMode.DoubleRow


def build_nc(N=1024):
    NT = N // P

    nc = bacc.Bacc(None, target_bir_lowering=False)
    with tile.TileContext(nc) as tc:
        with tc.tile_pool(name="dram", bufs=1, space="DRAM") as dram:
            xT8_d = dram.tile([DIM, N], E4, kind="ExternalInput")
            rxT8_d = dram.tile([DIM, N], E4, kind="ExternalInput")
            w8_d = dram.tile([DIM, F3], E4, kind="ExternalInput")
            rw8_d = dram.tile([DIM, F3], E4, kind="ExternalInput")
            wp_d = dram.tile([DIM, DIM], BF16, kind="ExternalInput")
            y_d = dram.tile([N, DIM], FP32, kind="ExternalOutput")
            _build_core(nc, tc, xT8_d, rxT8_d, w8_d, rw8_d, wp_d, y_d, N, NT)
    nc.compile()
    names = dict(xT8=xT8_d.name, rxT8=rxT8_d.name, w8=w8_d.name,
                 rw8=rw8_d.name, wp=wp_d.name, y=y_d.name)
    return nc, names


def _build_core(nc, tc, xT8_d, rxT8_d, w8_d, rw8_d, wp_d, y_d, N, NT):
    xT8_r = xT8_d[:].rearrange("(cs p) n -> p cs n", p=P)
    rxT8_r = rxT8_d[:].rearrange("(cs p) n -> p cs n", p=P)
    w8_r = w8_d[:].rearrange("(cs p) f -> p cs f", p=P)
    rw8_r = rw8_d[:].rearrange("(cs p) f -> p cs f", p=P)
    wp_r = wp_d[:].rearrange("(cs p) f -> p cs f", p=P)
    y_r = y_d[:].rearrange("(nt p) f -> p nt f", p=P)

    with (
        tc.tile_pool(name="consts", bufs=1) as consts,
        tc.tile_pool(name="persist", bufs=1) as persist,
        tc.tile_pool(name="qkt", bufs=2) as qkt_pool,
        tc.tile_pool(name="est", bufs=1) as est_pool,
        tc.tile_pool(name="onorm", bufs=16) as onorm_pool,
        tc.tile_pool(name="rden", bufs=4) as rden_pool,
        tc.tile_pool(name="ysb", bufs=2) as ysb_pool,
        tc.tile_pool(name="psum", bufs=1, space="PSUM") as psum,
    ):
        identb = consts.tile([P, P], BF16)
        make_identity(nc, identb[:])

        # persistent SBUF tensors
        xT8 = persist.tile([P, CS, N], E4)
        rxT8 = persist.tile([P, CS, N], E4)
        w8qk = persist.tile([P, CS, 2 * DIM], E4)
        rw8qk = persist.tile([P, CS, 2 * DIM], E4)
        wv8 = persist.tile([P, CS, DIM], E4)
        rwv8 = persist.tile([P, CS, DIM], E4)
        wp_sb = persist.tile([P, CS, DIM], BF16)
        V_sb = persist.tile([P, NT, H, HD + 1], BF16)
        OT = persist.tile([P, CS, N], BF16)

        # DMA order = first-use order: qk(0) runs first (xT8 + hp0-3 q/k
        # weight chunks + residuals), V' units follow as iteration-0 fill
        # (wv8), the other q/k chunks serve hp4-7, wp only feeds the tail.
        for cp in range(4):
            cs2 = slice(2 * cp, 2 * cp + 2)
            nc.sync.dma_start(xT8[:, cs2, :], xT8_r[:, cs2, :])
        for sl in (slice(0, 512), slice(DIM, DIM + 512)):
            nc.sync.dma_start(w8qk[:, :, sl], w8_r[:, :, sl])
        for cp in range(4):
            cs2 = slice(2 * cp, 2 * cp + 2)
            nc.sync.dma_start(rxT8[:, cs2, :], rxT8_r[:, cs2, :])
        for sl in (slice(0, 512), slice(DIM, DIM + 512)):
            nc.sync.dma_start(rw8qk[:, :, sl], rw8_r[:, :, sl])
        for cp in range(4):
            cs2 = slice(2 * cp, 2 * cp + 2)
            nc.sync.dma_start(wv8[:, cs2, :], w8_r[:, cs2, 2 * DIM:F3])
            nc.sync.dma_start(rwv8[:, cs2, :], rw8_r[:, cs2, 2 * DIM:F3])
        for sl in (slice(512, DIM), slice(DIM + 512, 2 * DIM)):
            nc.sync.dma_start(w8qk[:, :, sl], w8_r[:, :, sl])
            nc.sync.dma_start(rw8qk[:, :, sl], rw8_r[:, :, sl])
        nc.sync.dma_start(wp_sb[:], wp_r[:])
        nc.vector.memset(V_sb[:, :, :, HD:HD + 1], 1.0)

        # ---- V' = x @ Wv (+ones col), fp8 3-term -----------------------------
        # emitted as fill units inside the first iterations; term order
        # (x,w)x4, (rx,w)x4, (x,rw)x4 matches the DMA arrival order
        def vprime_unit(fc, nt):
            pv = psum.tile([P, 512], FP32, tag="u", bufs=2,
                           name=f"pv_{fc}_{nt}")
            xa_nt = xT8[:, :, nt * P:(nt + 1) * P]
            rx_nt = rxT8[:, :, nt * P:(nt + 1) * P]
            for g in range(2):
                out = pv[:, g * 256:(g + 1) * 256]
                f0 = fc * 512 + g * 256
                first = True
                for xa, wa in ((xa_nt, wv8), (rx_nt, wv8),
                               (xa_nt, rwv8)):
                    for i in range(4):
                        cp = slice(2 * i, 2 * i + 2)
                        nc.tensor.matmul(
                            out, xa[:, cp, :], wa[:, cp, f0:f0 + 256],
                            start=first,
                            stop=(i == 3 and wa is rwv8),
                            perf_mode=DR,
                        )
                        first = False
            nc.vector.tensor_scalar_mul(
                V_sb[:, nt, fc * 8:(fc + 1) * 8, 0:HD],
                pv[:, :].rearrange("p (h d) -> p h d", d=HD),
                INV_SXSW,
            )

        vprime_units = [(3072, (lambda fc_, nt_: lambda: vprime_unit(fc_, nt_))(fc, nt))
                        for fc in range(2) for nt in range(NT)]

        # ---- q/k projection (fp8 3-term), one psum tile per unit -----------
        def qk_half(hp, qk_t, qi, qc, g, state):
            ft = hp if qi == 0 else CS + hp
            if g == 0:
                state["pqk"] = psum.tile([P, 512], FP32, tag="u", bufs=2,
                                         name=f"pqk_{hp}_{qi}_{qc}")
            pqk = state["pqk"]
            out = pqk[:, g * 256:(g + 1) * 256]
            n0 = qc * 512 + g * 256
            first = True
            for wa, xa in ((w8qk, xT8), (w8qk, rxT8),
                           (rw8qk, xT8)):
                for i in range(4):
                    cp = slice(2 * i, 2 * i + 2)
                    nc.tensor.matmul(
                        out, wa[:, cp, ft * P:(ft + 1) * P],
                        xa[:, cp, n0:n0 + 256],
                        start=first,
                        stop=(i == 3 and wa is rw8qk),
                        perf_mode=DR,
                    )
                    first = False
            if g == 1:
                nc.vector.tensor_scalar_mul(
                    qk_t[:, qi, qc * 512:(qc + 1) * 512], pqk[:],
                    INV_SXSW)

        def qk_unit(hp, qk_t, qi, qc):
            state = {}
            for g in range(2):
                qk_half(hp, qk_t, qi, qc, g, state)

        def emit_qk_proj(hp):
            qk_t = qkt_pool.tile([P, 2, N], FP32R, tag="qkt",
                                 name=f"qk_t_{hp}")
            for qi in range(2):
                for qc in range(2):
                    qk_unit(hp, qk_t, qi, qc)
            return qk_t

        def s_unit(hp, qk_t, est, hi, kt):
            po = hi * HD
            ps = psum.tile([P, N], FP32, tag="s", bufs=2,
                           name=f"ps_{hp}_{hi}_{kt}")
            lhsT = qk_t[po:po + HD, 1, kt * P:(kt + 1) * P]
            for qc in range(2):
                nc.tensor.matmul(
                    ps[:, qc * 512:(qc + 1) * 512],
                    lhsT,
                    qk_t[po:po + HD, 0, qc * 512:(qc + 1) * 512],
                    start=True, stop=True,
                    tile_position=(po, 0),
                )
            nc.scalar.activation(est[hi][:, kt, :], ps[:], Exp, scale=SCALE)

        def make_est(hp):
            return [est_pool.tile([P, NT, N], BF16, tag=f"est{hi}",
                                  bufs=(2 if hi == 0 else 1),
                                  name=f"est_{hp}_{hi}")
                    for hi in range(2)]

        def pv_unit(hp, est, hi, qt, ons):
            h = 2 * hp + hi
            oacc = psum.tile([P, HD + 1], FP32, tag="o", bufs=2,
                             name=f"oacc_{hp}_{hi}_{qt}")
            for kt in range(NT):
                nc.tensor.matmul(
                    oacc[:],
                    est[hi][:, kt, qt * P:(qt + 1) * P],
                    V_sb[:, kt, h, :],
                    start=(kt == 0), stop=(kt == NT - 1),
                    skip_group_check=True,
                )
            rd = rden_pool.tile([P, 1], FP32, tag="rd",
                                name=f"rd_{hp}_{hi}_{qt}")
            nc.vector.reciprocal(rd[:], oacc[:, HD:HD + 1])
            nc.gpsimd.tensor_scalar_mul(
                ons[qt][:, hi, :], oacc[:, 0:HD], rd[:])

        def tr_unit(hp, ons, qt):
            tr = psum.tile([P, P], BF16, tag="o", bufs=2,
                           name=f"tr_{hp}_{qt}")
            nc.tensor.transpose(tr[:], ons[qt][:], identb[:])
            nc.gpsimd.tensor_copy(OT[:, hp, qt * P:(qt + 1) * P], tr[:])

        def make_ons(hp):
            return [onorm_pool.tile([P, 2, HD], BF16, tag="on",
                                    name=f"on_{hp}_{qt}")
                    for qt in range(NT)]

        def interleave(s_emits, fill):
            """Emit the 16 S+exp slots of an iteration, draining ~1467 PE
            cycles of fill work between consecutive slots so the PE stays
            busy while exps free the S psum buffers at their own pace."""
            SLOT = 1467.0
            fi = 0
            debt = 0.0
            for k, su in enumerate(s_emits):
                su()
                if k == 0:
                    continue
                debt += SLOT
                while fi < len(fill) and debt > 0:
                    c, f = fill[fi]
                    f()
                    debt -= c
                    fi += 1
            for c, f in fill[fi:]:
                f()

        # Pipeline: iteration hp interleaves S(hp)+exp(hp) slots with
        # PV/transpose work for hp-1 and the q/k projection for hp+1.
        import functools
        qk_t = emit_qk_proj(0)
        est_prev = ons_prev = None
        for hp in range(CS):
            est = make_est(hp)
            s_emits = [functools.partial(s_unit, hp, qk_t, est, hi, kt)
                       for hi in range(2) for kt in range(NT)]
            fill = []
            if hp >= 1:
                ons = make_ons(hp - 1)
                fill += [(520, functools.partial(pv_unit, hp - 1, est_prev,
                                                 hi, qt, ons))
                         for hi in range(2) for qt in range(NT)]
            if hp + 1 < CS:
                qk_next = qkt_pool.tile([P, 2, N], FP32R, tag="qkt",
                                        name=f"qk_t_{hp + 1}")
                qk_states = {(qi, qc): {} for qi in range(2)
                             for qc in range(2)}
                fill += [(1536, functools.partial(qk_half, hp + 1, qk_next,
                                                  qi, qc, g,
                                                  qk_states[(qi, qc)]))
                         for qi in range(2) for qc in range(2)
                         for g in range(2)]
            else:
                qk_next = None
            if hp == 0:
                fill += vprime_units
            if hp >= 1:
                fill += [(128, functools.partial(tr_unit, hp - 1, ons, qt))
                         for qt in range(NT)]
            if hp == CS - 1:
                ons_last = make_ons(hp)
                fill += [(520, functools.partial(pv_unit, hp, est, 0, qt,
                                                 ons_last))
                         for qt in range(NT)]
            interleave(s_emits, fill)
            est_prev = est
            qk_t = qk_next
        for qt in range(NT):
            pv_unit(CS - 1, est_prev, 1, qt, ons_last)
        tr_unit(CS - 1, ons_last, 0)
        ons_prev = ons_last

        # ---- y = OT^T @ wp (bf16) -------------------------------------------
        # chained with the last head-pair's transposes: proj(nt) only needs
        # OT[:, 7, nt-chunk], so each tr(7, qt) immediately unblocks one
        # projection tile. 2-bank psum per nt (tag "s" is idle by now).
        for nt in range(NT):
            if nt + 1 < NT:
                tr_unit(CS - 1, ons_prev, nt + 1)
            py = psum.tile([P, DIM], FP32, tag="s", bufs=2,
                           name=f"py_{nt}")
            for cs in range(CS):
                lhsT = OT[:, cs, nt * P:(nt + 1) * P]
                for fc in range(2):
                    nc.tensor.matmul(
                        py[:, fc * 512:(fc + 1) * 512],
                        lhsT, wp_sb[:, cs, fc * 512:(fc + 1) * 512],
                        start=(cs == 0), stop=(cs == CS - 1),
                    )
            y_sb = ysb_pool.tile([P, DIM], FP32, tag="ysb",
                                 name=f"y_sb_{nt}")
            for fc in range(2):
                nc.vector.tensor_copy(y_sb[:, fc * 512:(fc + 1) * 512],
                                      py[:, fc * 512:(fc + 1) * 512])
                nc.sync.dma_start(y_r[:, nt, fc * 512:(fc + 1) * 512],
                                  y_sb[:, fc * 512:(fc + 1) * 512])


_CACHE = {}


def _get_nc(N=1024):
    if N not in _CACHE:
        _CACHE[N] = build_nc(N)
    return _CACHE[N]


E4NP = ml_dtypes.float8_e4m3


def _prep_weights(w_qkv, w_proj):
    w_qkv = np.asarray(w_qkv, np.float32)
    w_proj = np.asarray(w_proj, np.float32)
    w8 = (w_qkv * SW).astype(E4NP)
    rw8 = (w_qkv * SW - w8.astype(np.float32)).astype(E4NP)
    # permute rows c = d*16+h -> c' = h*64+d to undo the reference's
    # [B,N,hd,H] output interleave
    wp_perm = np.ascontiguousarray(
        w_proj.reshape(HD, H, DIM).transpose(1, 0, 2).reshape(DIM, DIM))
    wp_bf = wp_perm.astype(ml_dtypes.bfloat16)
    return np.ascontiguousarray(w8), np.ascontiguousarray(rw8), wp_bf


def kernel(x, w_qkv, w_proj, b_proj):
    """Full inputs in, full output out. Shards batch across 8 cores."""
    from concourse.bass_utils import run_bass_kernel_spmd

    B, N, C = x.shape
    assert (B, C) == (8, DIM)
    nc, nm = _get_nc(N)
    x = np.asarray(x, dtype=np.float32)
    w8, rw8, wp_bf = _prep_weights(w_qkv, w_proj)
    b_proj_np = np.asarray(b_proj, dtype=np.float32).reshape(1, DIM)

    in_maps = []
    for b in range(B):
        xT = np.ascontiguousarray(x[b].T)
        xT8 = (xT * SX).astype(E4NP)
        rxT8 = (xT * SX - xT8.astype(np.float32)).astype(E4NP)
        in_maps.append({nm["xT8"]: xT8, nm["rxT8"]: rxT8, nm["w8"]: w8,
                       nm["rw8"]: rw8, nm["wp"]: wp_bf})
    res = run_bass_kernel_spmd(nc, in_maps, core_ids=list(range(8)))
    y = np.stack([res.results[b][nm["y"]] for b in range(B)], axis=0)
    if np.any(b_proj_np):
        # exact host-side bias add; no-op for the zero bias this model ships
        y = (y + b_proj_np.reshape(1, 1, DIM)).astype(np.float32)
    return y
